# revision 1
# baseline (speedup 1.0000x reference)
"""Trainium2 Bass kernel for nn_Cheb_35888746725726 (ChebConv K=3 GNN, N=50000,
E=800000, F=H=96, lambda_max=2 -> diag term is 0).

Strategy (8 NeuronCores, node/graph-parallel):
 - Host: compute Chebyshev edge norm (deg/rsqrt/norm_w), degree-balanced node
   permutation into 784 tiles of 64 nodes (padded N=50176 = 8 cores x 6272),
   shard edges by destination core, sort per 64-node dst tile, pad each tile's
   edge list to 8x128 slots. Precompute per-edge-tile *weighted one-hot*
   scatter matrices (128 edges x 64 dst-slots, bf16, norm_w folded in) which
   stay resident in SBUF for all 4 propagations.
 - Device per prop: indirect-DMA gather of source rows (bf16) from the HBM
   node-feature table -> scatter via one-hot matmuls accumulating in PSUM.
   Chebyshev recurrence folded into host-modified dense weights:
   out = Tx0 @ (W0-W2) + Tx1 @ W1 + (L@Tx1) @ (2*W2), so Tx2 is never formed.
 - AllGather (8 cores) rebuilds the full node table between dependent props.
 - Dense 96x96 matmuls run feature-major; PE transposes convert layouts.
"""
import numpy as np
import ml_dtypes

import concourse.bass as bass
import concourse.bacc as bacc
import concourse.mybir as mybir
import concourse.tile as tile
from concourse.bass_utils import run_bass_kernel_spmd
from concourse.masks import make_identity

# ---- problem constants (hardcoded per the harness contract) ----
N = 50000
E = 800000
F = 96
K = 3
C = 8                    # cores
NP_PAD = 50176           # 8 * 6272
SHARD = NP_PAD // C      # 6272
NTW = 64                 # node-tile width
NT = SHARD // NTW        # 98 node tiles / core
TE = 8                   # edge tiles (of 128 edges) per node tile
P = 128
NCHUNK = 512             # dense matmul node-chunk
G = 7                    # node tiles per gather call (98 = 14 * 7)

BF = ml_dtypes.bfloat16

import os
DBG_NO_AG = bool(int(os.environ.get("KDBG_NO_AG", "0")))     # replace collectives with local copies
DBG_CORES = int(os.environ.get("KDBG_CORES", str(C)))         # cores to run
DBG_TE = int(os.environ.get("KDBG_TE", str(TE)))              # edge tiles per node tile (perf bisect)

_compiled = None         # cache (nc, meta) across calls


# --------------------------------------------------------------------------
# host-side preprocessing
# --------------------------------------------------------------------------
def _preprocess(x, edge_index, edge_weight):
    src = np.asarray(edge_index[0]).astype(np.int64)
    dst = np.asarray(edge_index[1]).astype(np.int64)
    w = np.asarray(edge_weight).astype(np.float32)

    deg = np.zeros(N, np.float32)
    np.add.at(deg, src, w)
    dis = np.where(deg > 0, 1.0 / np.sqrt(np.maximum(deg, 1e-30)), 0.0).astype(np.float32)
    norm_w = (-dis[src] * w * dis[dst]).astype(np.float32)

    # degree-balanced assignment of nodes to 784 tiles of 64 (LPT greedy)
    indeg = np.bincount(dst, minlength=N).astype(np.int64)
    n_tiles = NP_PAD // NTW
    order = np.argsort(-indeg, kind="stable")
    import heapq
    heap = [(0, 0, t) for t in range(n_tiles)]
    heapq.heapify(heap)
    tile_assign = np.empty(N, np.int64)
    pending = []
    for n in order:
        while True:
            load, cnt, t = heapq.heappop(heap)
            if cnt < NTW:
                tile_assign[n] = t
                heapq.heappush(heap, (load + indeg[n], cnt + 1, t))
                break
            # full tile: drop from heap permanently
    new_id = np.full(N, -1, np.int64)
    slot = np.zeros(n_tiles, np.int64)
    for n in range(N):
        t = tile_assign[n]
        new_id[n] = t * NTW + slot[t]
        slot[t] += 1

    src_n = new_id[src]
    dst_n = new_id[dst]

    tile_load = np.zeros(n_tiles, np.int64)
    np.add.at(tile_load, dst_n // NTW, 1)
    assert tile_load.max() <= TE * P, f"tile overflow: {tile_load.max()}"

    # per-core edge structures
    src_idx = np.zeros((C, P, NT * TE), np.int32)
    oh = np.zeros((C, P, NT * TE * NTW), BF)
    for c in range(C):
        m = (dst_n // SHARD) == c
        es = src_n[m]
        ed = dst_n[m] - c * SHARD
        ew = norm_w[m]
        o = np.argsort(ed, kind="stable")
        es, ed, ew = es[o], ed[o], ew[o]
        tile_of = ed // NTW
        # bucket boundaries per node tile
        starts = np.searchsorted(tile_of, np.arange(NT))
        ends = np.searchsorted(tile_of, np.arange(NT) + 1)
        oh_c = np.zeros((NT * TE, P, NTW), np.float32)
        for nt in range(NT):
            s0, s1 = starts[nt], ends[nt]
            cnt = s1 - s0
            sl = np.arange(cnt)
            t_i = sl // P
            p_i = sl % P
            src_idx[c, p_i, nt * TE + t_i] = es[s0:s1]
            oh_c[nt * TE + t_i, p_i, ed[s0:s1] - nt * NTW] = ew[s0:s1]
        oh[c] = oh_c.astype(BF).transpose(1, 0, 2).reshape(P, NT * TE * NTW)

    return new_id, src_idx, oh


# --------------------------------------------------------------------------
# bass kernel builder
# --------------------------------------------------------------------------
def _build_kernel():
    dt = mybir.dt
    nc = bacc.Bacc("TRN2", target_bir_lowering=False, debug=False, num_devices=DBG_CORES)

    x_tab = nc.dram_tensor("x_tab", [NP_PAD, F], dt.bfloat16, kind="ExternalInput")
    src_d = nc.dram_tensor("src_idx", [P, NT * TE], dt.int32, kind="ExternalInput")
    oh_d = nc.dram_tensor("oh", [P, NT * TE * NTW], dt.bfloat16, kind="ExternalInput")
    xT_d = nc.dram_tensor("xT_own", [F, SHARD], dt.bfloat16, kind="ExternalInput")
    w_d = nc.dram_tensor("wmats", [6 * F, F], dt.bfloat16, kind="ExternalInput")
    wlin_d = nc.dram_tensor("wlin", [F, 2], dt.bfloat16, kind="ExternalInput")
    bias_d = nc.dram_tensor("biases", [F, 2], dt.float32, kind="ExternalInput")  # b1,b2 cols
    blin_d = nc.dram_tensor("blin", [2, 1], dt.float32, kind="ExternalInput")
    out_d = nc.dram_tensor("out", [2, SHARD], dt.float32, kind="ExternalOutput")

    rg = [list(range(C))]

    with tile.TileContext(nc) as tc:
        with (
            tc.tile_pool(name="res", bufs=1) as res,          # resident sbuf
            tc.tile_pool(name="mpool", bufs=4) as mpool,      # gather dests
            tc.tile_pool(name="spool", bufs=2) as spool,      # small evac tiles
            tc.tile_pool(name="pscat", bufs=4, space="PSUM") as pscat,
            tc.tile_pool(name="ptr", bufs=2, space="PSUM") as ptr,
            tc.tile_pool(name="pdense", bufs=2, space="PSUM") as pdense,
            tc.tile_pool(name="dram", bufs=1, space="DRAM") as dram,
        ):
            # ---------- resident loads ----------
            oh_sb = res.tile([P, NT * TE * NTW], dt.bfloat16)
            n_oh_chunks = 14
            csz = NT * TE * NTW // n_oh_chunks
            for i in range(n_oh_chunks):
                nc.sync.dma_start(out=oh_sb[:, i * csz:(i + 1) * csz],
                                  in_=oh_d[:, i * csz:(i + 1) * csz])
            src_sb = res.tile([P, NT * TE], dt.int32)
            nc.sync.dma_start(out=src_sb[:], in_=src_d[:])
            w_sb = res.tile([F, 6 * F], dt.bfloat16)   # 6 lhsT mats side by side
            for i in range(6):
                nc.sync.dma_start(out=w_sb[:, i * F:(i + 1) * F],
                                  in_=w_d[i * F:(i + 1) * F, :])
            wlin_sb = res.tile([F, 2], dt.bfloat16)
            nc.sync.dma_start(out=wlin_sb[:], in_=wlin_d[:])
            bias_sb = res.tile([F, 2], dt.float32)
            nc.sync.dma_start(out=bias_sb[:], in_=bias_d[:])
            blin_sb = res.tile([2, 1], dt.float32)
            nc.sync.dma_start(out=blin_sb[:], in_=blin_d[:])
            ident = res.tile([P, P], dt.bfloat16)
            make_identity(nc, ident[:])

            # feature-major activation buffers (bf16)
            fm = {
                "tx0": res.tile([F, SHARD], dt.bfloat16, name="fm_tx0"),
                "t1": res.tile([F, SHARD], dt.bfloat16, name="fm_t1"),
                "s2": res.tile([F, SHARD], dt.bfloat16, name="fm_s2"),
                "h": res.tile([F, SHARD], dt.bfloat16, name="fm_h"),
            }
            nc.sync.dma_start(out=fm["tx0"][:], in_=xT_d[:])

            # node-major staging for table writes / transposes
            s_nm = res.tile([P, (NT // 2) * F], dt.bfloat16)

            # internal DRAM
            bounce = [dram.tile([SHARD, F], dt.bfloat16, name=f"bounce{i}") for i in range(3)]
            ag = [dram.tile([NP_PAD, F], dt.bfloat16,
                            addr_space=("Local" if DBG_NO_AG else "Shared"), name=f"ag{i}")
                  for i in range(3)]

            # ---------- helpers ----------
            def prop(table_ap, tag, probe=False):
                """one propagation: gather+scatter; results land in s_nm (node-major)."""
                with nc.named_scope(f"prop_{tag}"):
                    pr = None
                    if probe:
                        # tiny gpsimd-issued DMA touching the table: executes the
                        # collective-completion wait so the 1-wait-limited
                        # dynamic gathers below don't need it
                        pr = spool.tile([1, 2], dt.bfloat16, tag="pr")
                        nc.gpsimd.dma_start(out=pr[:], in_=table_ap.tensor[0:1, 0:2])
                    for nt in range(NT):
                        m_t = mpool.tile([P, TE * F], dt.bfloat16, tag="m")
                        # absorber: one strided gpsimd write touching each edge
                        # tile's corner carries the slot's WAR/WAW waits (and the
                        # table-probe dep for the first tile) so each 1-wait-
                        # limited dynamic gather below needs at most one wait.
                        # HW note: indirect DMA honors only ONE offset column
                        # per call, hence one gather per 128-edge tile.
                        if pr is not None and nt == 0:
                            nc.gpsimd.tensor_copy(out=m_t[0:1, 0:1], in_=pr[0:1, 0:1])
                        corner = m_t[:].rearrange("p (t f) -> p t f", f=F)[0:1, :, 0:1]
                        nc.gpsimd.memset(corner, 0)
                        for t in range(DBG_TE):
                            nc.gpsimd.indirect_dma_start(
                                out=m_t[:, t * F:(t + 1) * F],
                                out_offset=None,
                                in_=table_ap,
                                in_offset=bass.IndirectOffsetOnAxis(
                                    ap=src_sb[:, nt * TE + t:nt * TE + t + 1], axis=0),
                            )
                        ps = pscat.tile([NTW, F], dt.float32, space="PSUM", tag="ps")
                        for t in range(DBG_TE):
                            nc.tensor.matmul(
                                out=ps[:],
                                lhsT=oh_sb[:, (nt * TE + t) * NTW:(nt * TE + t + 1) * NTW],
                                rhs=m_t[:, t * F:(t + 1) * F],
                                start=(t == 0),
                                stop=(t == DBG_TE - 1),
                            )
                        j, b = nt // 2, nt % 2
                        nc.vector.tensor_copy(
                            out=s_nm[b * NTW:(b + 1) * NTW, j * F:(j + 1) * F],
                            in_=ps[:])

            def table_write_and_ag(idx):
                """write s_nm -> bounce[idx] (node-major [SHARD, F]) and allgather."""
                with nc.named_scope(f"ag_{idx}"):
                    bo = bounce[idx]
                    view = bo[:].rearrange("(j p) f -> p j f", p=P)
                    nc.sync.dma_start(out=view, in_=s_nm[:].rearrange("p (j f) -> p j f", f=F))
                    if DBG_NO_AG:
                        for r in range(C):
                            nc.sync.dma_start(out=ag[idx][r * SHARD:(r + 1) * SHARD, :],
                                              in_=bo[:])
                    else:
                        nc.gpsimd.collective_compute(
                            "AllGather",
                            mybir.AluOpType.bypass,
                            replica_groups=rg,
                            ins=[bo.opt()],
                            outs=[ag[idx].opt()],
                        )

            def snm_to_fm(dest, tag):
                """transpose node-major s_nm into feature-major dest tile."""
                with nc.named_scope(f"fm_{tag}"):
                    for j in range(NT // 2):
                        pt = ptr.tile([F, P], dt.bfloat16, space="PSUM", tag="pt")
                        nc.tensor.transpose(out=pt[:], in_=s_nm[:, j * F:(j + 1) * F],
                                            identity=ident[:])
                        nc.vector.tensor_copy(out=dest[:, j * P:(j + 1) * P], in_=pt[:])

            def fm_to_snm(src_t, tag):
                """transpose feature-major tile back into s_nm node-major staging."""
                with nc.named_scope(f"nm_{tag}"):
                    for j in range(NT // 2):
                        pt = ptr.tile([P, F], dt.bfloat16, space="PSUM", tag="pt")
                        nc.tensor.transpose(out=pt[:], in_=src_t[:, j * P:(j + 1) * P],
                                            identity=ident[:F, :F])
                        nc.vector.tensor_copy(out=s_nm[:, j * F:(j + 1) * F], in_=pt[:])

            def dense(layer, tx0_t, t1_t, s2_t, h_t):
                """h = relu(tx0@W0' + t1@W1 + s2@W2') feature-major, bf16 out."""
                with nc.named_scope(f"dense_{layer}"):
                    wof = layer * 3 * F
                    nchunks = (SHARD + NCHUNK - 1) // NCHUNK
                    for ci in range(nchunks):
                        c0 = ci * NCHUNK
                        c1 = min(SHARD, c0 + NCHUNK)
                        pd = pdense.tile([F, NCHUNK], dt.float32, space="PSUM", tag="pd")
                        for ki, rhs_t in enumerate((tx0_t, t1_t, s2_t)):
                            nc.tensor.matmul(
                                out=pd[:, :c1 - c0],
                                lhsT=w_sb[:, wof + ki * F:wof + (ki + 1) * F],
                                rhs=rhs_t[:, c0:c1],
                                start=(ki == 0),
                                stop=(ki == 2),
                            )
                        nc.scalar.activation(
                            out=h_t[:, c0:c1], in_=pd[:, :c1 - c0],
                            func=mybir.ActivationFunctionType.Relu,
                            bias=bias_sb[:, layer:layer + 1],
                        )

            # ---------- pipeline ----------
            obs_t = res.tile([1, 1], dt.int32)
            nc.gpsimd.tensor_copy(out=obs_t[:], in_=src_sb[0:1, 0:1])

            # Layer 1
            prop(x_tab[:], "l1a")                     # s_nm = Tx1 own (node-major)
            table_write_and_ag(0)                     # ag[0] = Tx1 full
            snm_to_fm(fm["t1"], "t1")
            prop(ag[0][:], "l1b", probe=True)                     # s_nm = L@Tx1 own
            snm_to_fm(fm["s2"], "s2")
            dense(0, fm["tx0"], fm["t1"], fm["s2"], fm["h"])
            fm_to_snm(fm["h"], "h1")
            table_write_and_ag(1)                     # ag[1] = h1 full

            # Layer 2
            prop(ag[1][:], "l2a", probe=True)
            table_write_and_ag(2)                     # ag[2] = Tx1' full
            snm_to_fm(fm["t1"], "t1b")
            prop(ag[2][:], "l2b", probe=True)
            snm_to_fm(fm["s2"], "s2b")
            dense(1, fm["h"], fm["t1"], fm["s2"], fm["tx0"])   # h2 -> fm["tx0"]

            # final linear [2 x SHARD]
            with nc.named_scope("final"):
                nchunks = (SHARD + NCHUNK - 1) // NCHUNK
                for ci in range(nchunks):
                    c0 = ci * NCHUNK
                    c1 = min(SHARD, c0 + NCHUNK)
                    pf = pdense.tile([2, NCHUNK], dt.float32, space="PSUM", tag="pd")
                    nc.tensor.matmul(out=pf[:, :c1 - c0], lhsT=wlin_sb[:],
                                     rhs=fm["tx0"][:, c0:c1], start=True, stop=True)
                    ot = spool.tile([2, NCHUNK], dt.float32, tag="ot")
                    nc.scalar.activation(
                        out=ot[:, :c1 - c0], in_=pf[:, :c1 - c0],
                        func=mybir.ActivationFunctionType.Identity,
                        bias=blin_sb[:],
                    )
                    nc.sync.dma_start(out=out_d[:, c0:c1], in_=ot[:, :c1 - c0])

    nc.compile()
    return nc


# --------------------------------------------------------------------------
# entry point
# --------------------------------------------------------------------------
def kernel(x, edge_index, edge_weight, W1, b1, W2, b2, Wlin, blin,
           _trace=False, _tmpdir=None):
    global _compiled
    x = np.asarray(x, np.float32)
    W1 = np.asarray(W1, np.float32); W2 = np.asarray(W2, np.float32)
    b1 = np.asarray(b1, np.float32); b2 = np.asarray(b2, np.float32)
    Wlin = np.asarray(Wlin, np.float32); blin = np.asarray(blin, np.float32)

    new_id, src_idx, oh = _preprocess(x, edge_index, edge_weight)

    # padded permuted node table (bf16)
    xp = np.zeros((NP_PAD, F), np.float32)
    xp[new_id] = x
    x_tab = xp.astype(BF)

    # folded dense weights: [W0-W2, W1, 2*W2] per layer
    wm = np.concatenate([
        W1[0] - W1[2], W1[1], 2.0 * W1[2],
        W2[0] - W2[2], W2[1], 2.0 * W2[2],
    ], axis=0).astype(BF)                       # [6F, F]
    biases = np.stack([b1, b2], axis=1).astype(np.float32)      # [F, 2]

    in_maps = []
    for c in range(C):
        xT_own = np.ascontiguousarray(xp[c * SHARD:(c + 1) * SHARD].T).astype(BF)
        in_maps.append({
            "x_tab": x_tab,
            "src_idx": src_idx[c],
            "oh": oh[c],
            "xT_own": xT_own,
            "wmats": wm,
            "wlin": Wlin.astype(BF),
            "biases": biases,
            "blin": blin.reshape(2, 1).astype(np.float32),
        })

    if _compiled is None:
        _compiled = _build_kernel()
    nc = _compiled

    import time as _time
    _t0 = _time.perf_counter()
    try:
        res = run_bass_kernel_spmd(nc, in_maps[:DBG_CORES], core_ids=list(range(DBG_CORES)),
                                   trace=_trace, tmpdir=_tmpdir)
    except ModuleNotFoundError:
        # axon NTFF hook unavailable in this container; run untraced
        res = run_bass_kernel_spmd(nc, in_maps[:DBG_CORES], core_ids=list(range(DBG_CORES)),
                                   trace=False, tmpdir=_tmpdir)
    kernel.last_spmd_wall_s = _time.perf_counter() - _t0

    outs_per_core = [np.asarray(res.results[c]["out"]) for c in range(len(res.results))]
    while len(outs_per_core) < C:
        outs_per_core.append(outs_per_core[-1])
    out_p = np.concatenate(outs_per_core, axis=1)   # [2, NP_PAD]
    out = out_p.T[new_id].astype(np.float32)    # [N, 2]
    if _trace:
        kernel.last_exec_time_ns = res.exec_time_ns
        kernel.last_results = res
    return out



# revision 6
# speedup vs baseline: 9.0932x; 9.0932x over previous
"""Trainium2 Bass kernel for nn_Cheb_35888746725726 (ChebConv K=3 GNN, N=50000,
E=800000, F=H=96, lambda_max=2 -> diag term is 0).

Strategy (8 NeuronCores, node/graph-parallel). The wall-clock of a call is
dominated by host->device transfer over the axon tunnel (~77MB/s + ~35ms fixed
per array), so inputs are packed into TWO uint16 blobs per core (~1.9MB/core
total) and everything derivable is rebuilt on device:
 - x ships once, sharded node-major; an AllGather rebuilds the full node table.
 - The per-edge-tile weighted one-hot scatter matrices (128 edges x 64 dst
   slots) are built on device from compact (slot, weight) pairs via DVE
   is_equal+mult against an iota row.
 - Feature-major x comes from on-device PE transposes.
 - Gather indices ship as uint16 and are cast-copied to int32 on device.
Device pipeline per prop: indirect-DMA gather of source rows (bf16) from the
HBM node table -> scatter via one-hot matmuls accumulating in PSUM. Chebyshev
recurrence folded into host-modified dense weights:
  out = Tx0 @ (W0-W2) + Tx1 @ W1 + (L@Tx1) @ (2*W2), so Tx2 is never formed.
AllGather (8 cores) rebuilds the full node table between dependent props.
The PJRT executable is compiled once and cached; per-call work is vectorized
numpy preprocessing overlapped with the async device_put of the blobs.
"""
import numpy as np
import ml_dtypes

import concourse.bass as bass
import concourse.bacc as bacc
import concourse.mybir as mybir
import concourse.tile as tile
from concourse.masks import make_identity

# ---- problem constants (hardcoded per the harness contract) ----
N = 50000
E = 800000
F = 96
K = 3
C = 8                    # cores
NP_PAD = 50176           # 8 * 6272
SHARD = NP_PAD // C      # 6272
NTW = 64                 # node-tile width
NT = SHARD // NTW        # 98 node tiles / core
TE = 8                   # edge tiles (of 128 edges) per node tile
P = 128
NCHUNK = 512             # dense matmul node-chunk
N_TILES = NP_PAD // NTW  # 784 node tiles globally

BF = ml_dtypes.bfloat16

# blob layouts (uint16 units)
XLEN = SHARD * F                     # 602112  : x_own bf16 [SHARD, F]
E_SLOT = 0                           # bf16 [P, NT*TE] dst slot within node tile
E_W = E_SLOT + P * NT * TE           # bf16 [P, NT*TE] edge weight
E_SRC = E_W + P * NT * TE            # uint16 [P, NT*TE] src node id
E_WM = E_SRC + P * NT * TE           # bf16 [6F, F] folded dense weights
E_WLIN = E_WM + 6 * F * F            # bf16 [F, 2]
E_B = E_WLIN + F * 2                 # f32 [F, 2] biases (b1, b2 cols)
E_BLIN = E_B + 2 * F * 2             # f32 [2]
ELEN = E_BLIN + 4

import os
DBG_NO_AG = bool(int(os.environ.get("KDBG_NO_AG", "0")))

_RT = None               # cached runtime: nc + jitted executable + metadata


# --------------------------------------------------------------------------
# host-side preprocessing (fully vectorized)
# --------------------------------------------------------------------------
def _norm_and_perm(src, dst, w):
    """Chebyshev edge norm + degree-balanced node permutation (snake)."""
    deg = np.bincount(src, weights=w.astype(np.float64), minlength=N)
    deg = deg.astype(np.float32)
    dis = np.where(deg > 0, 1.0 / np.sqrt(np.maximum(deg, 1e-30)), 0.0)
    dis = dis.astype(np.float32)
    norm_w = (-dis[src] * w * dis[dst]).astype(np.float32)

    # snake assignment of degree-sorted nodes -> balanced per-tile edge load
    indeg = np.bincount(dst, minlength=N)
    order = np.argsort(-indeg, kind="stable")
    r = np.arange(N)
    blk = r // N_TILES
    pos = r % N_TILES
    t_r = np.where(blk % 2 == 0, pos, N_TILES - 1 - pos)
    tile_assign = np.empty(N, np.int64)
    tile_assign[order] = t_r
    slot_assign = np.empty(N, np.int64)
    slot_assign[order] = blk

    # repair: per-tile edge load must fit TE*P scatter slots
    cap = TE * P
    tl = np.bincount(tile_assign[dst], minlength=N_TILES)
    for _ in range(2000):
        if tl.max() <= cap:
            break
        t_over = int(tl.argmax())
        t_under = int(tl.argmin())
        no = np.where(tile_assign == t_over)[0]
        nu = np.where(tile_assign == t_under)[0]
        a = no[np.argmax(indeg[no])]
        b = nu[np.argmin(indeg[nu])]
        tile_assign[a], tile_assign[b] = t_under, t_over
        slot_assign[a], slot_assign[b] = slot_assign[b], slot_assign[a]
        d = indeg[a] - indeg[b]
        tl[t_over] -= d
        tl[t_under] += d
    assert tl.max() <= cap, f"tile overflow after repair: {tl.max()}"

    new_id = tile_assign * NTW + slot_assign
    return norm_w, new_id


def _pack_edges(src_n, dst_n, norm_w):
    """Per-core compact edge blobs: slot/weight/src in [P, NT*TE] layout."""
    eb = np.zeros((C, ELEN), np.uint16)
    slot_v = eb[:, E_SLOT:E_W].reshape(C, P, NT * TE).view(BF)
    w_v = eb[:, E_W:E_SRC].reshape(C, P, NT * TE).view(BF)
    src_v = eb[:, E_SRC:E_WM].reshape(C, P, NT * TE)

    core = dst_n // SHARD
    for c in range(C):
        m = core == c
        ed = dst_n[m] - c * SHARD
        es = src_n[m]
        ew = norm_w[m]
        o = np.argsort(ed, kind="stable")
        ed, es, ew = ed[o], es[o], ew[o]
        tof = ed >> 6
        starts = np.searchsorted(tof, np.arange(NT))
        rank = np.arange(ed.size) - starts[tof]
        p_i = rank & (P - 1)
        col = tof * TE + (rank >> 7)
        src_v[c, p_i, col] = es.astype(np.uint16)
        slot_v[c, p_i, col] = (ed & 63).astype(np.float32)
        w_v[c, p_i, col] = ew
    return eb


def _fold_weights(W1, b1, W2, b2, Wlin, blin, eb):
    wm = np.concatenate([
        W1[0] - W1[2], W1[1], 2.0 * W1[2],
        W2[0] - W2[2], W2[1], 2.0 * W2[2],
    ], axis=0).astype(BF)                                     # [6F, F]
    eb[:, E_WM:E_WLIN] = wm.reshape(-1).view(np.uint16)[None, :]
    eb[:, E_WLIN:E_B] = Wlin.astype(BF).reshape(-1).view(np.uint16)[None, :]
    biases = np.stack([b1, b2], axis=1).astype(np.float32)    # [F, 2]
    eb[:, E_B:E_BLIN] = biases.reshape(-1).view(np.uint16)[None, :]
    eb[:, E_BLIN:ELEN] = blin.astype(np.float32).reshape(-1).view(np.uint16)[None, :]


# --------------------------------------------------------------------------
# bass kernel builder
# --------------------------------------------------------------------------
def _build_kernel():
    dt = mybir.dt
    nc = bacc.Bacc("TRN2", target_bir_lowering=False, debug=False, num_devices=C)

    bx_d = nc.dram_tensor("bx", [XLEN], dt.uint16, kind="ExternalInput")
    be_d = nc.dram_tensor("be", [ELEN], dt.uint16, kind="ExternalInput")
    out_d = nc.dram_tensor("out", [2, SHARD], dt.float32, kind="ExternalOutput")

    bx_bf = bx_d.bitcast(dt.bfloat16)
    be_bf = be_d.bitcast(dt.bfloat16)
    be_f32 = be_d.bitcast(dt.float32)

    rg = [list(range(C))]

    with tile.TileContext(nc) as tc:
        with (
            tc.tile_pool(name="res", bufs=1) as res,          # resident sbuf
            tc.tile_pool(name="mpool", bufs=4) as mpool,      # gather dests
            tc.tile_pool(name="spool", bufs=2) as spool,      # small evac tiles
            tc.tile_pool(name="pscat", bufs=4, space="PSUM") as pscat,
            tc.tile_pool(name="ptr", bufs=2, space="PSUM") as ptr,
            tc.tile_pool(name="pdense", bufs=2, space="PSUM") as pdense,
            tc.tile_pool(name="dram", bufs=1, space="DRAM") as dram,
        ):
            # ---------- resident loads (from the packed blobs) ----------
            # slot/weight land as f32 (cast during SWDGE DMA): tensor_scalar
            # is_equal requires float32 scalar operands
            slot_sb = res.tile([P, NT * TE], dt.float32)
            nc.gpsimd.dma_start(out=slot_sb[:],
                                in_=be_bf[E_SLOT:E_W].rearrange("(p c) -> p c", p=P))
            we_sb = res.tile([P, NT * TE], dt.float32)
            nc.gpsimd.dma_start(out=we_sb[:],
                                in_=be_bf[E_W:E_SRC].rearrange("(p c) -> p c", p=P))
            srcu_sb = res.tile([P, NT * TE], dt.uint16)
            nc.sync.dma_start(out=srcu_sb[:],
                              in_=be_d[E_SRC:E_WM].rearrange("(p c) -> p c", p=P))
            src_sb = res.tile([P, NT * TE], dt.int32)
            nc.vector.tensor_copy(out=src_sb[:], in_=srcu_sb[:])

            w_sb = res.tile([F, 6 * F], dt.bfloat16)   # 6 lhsT mats side by side
            for i in range(6):
                nc.sync.dma_start(
                    out=w_sb[:, i * F:(i + 1) * F],
                    in_=be_bf[E_WM + i * F * F:E_WM + (i + 1) * F * F]
                        .rearrange("(a b) -> a b", b=F))
            wlin_sb = res.tile([F, 2], dt.bfloat16)
            nc.sync.dma_start(out=wlin_sb[:],
                              in_=be_bf[E_WLIN:E_B].rearrange("(f b) -> f b", b=2))
            bias_sb = res.tile([F, 2], dt.float32)
            nc.sync.dma_start(out=bias_sb[:],
                              in_=be_f32[E_B // 2:E_BLIN // 2]
                                  .rearrange("(f b) -> f b", b=2))
            blin_sb = res.tile([2, 1], dt.float32)
            nc.sync.dma_start(out=blin_sb[:],
                              in_=be_f32[E_BLIN // 2:ELEN // 2]
                                  .rearrange("(p o) -> p o", o=1))
            ident = res.tile([P, P], dt.bfloat16)
            make_identity(nc, ident[:])

            # one-hot scatter matrices built on device: oh[p, e*64+s] =
            # (s == slot[p,e]) * w[p,e]
            iota_i = res.tile([P, NTW], dt.int32)
            nc.gpsimd.iota(iota_i[:], pattern=[[1, NTW]], base=0,
                           channel_multiplier=0)
            iota_bf = res.tile([P, NTW], dt.bfloat16)
            nc.vector.tensor_copy(out=iota_bf[:], in_=iota_i[:])
            oh_sb = res.tile([P, NT * TE * NTW], dt.bfloat16)
            for e in range(NT * TE):
                nc.vector.tensor_scalar(
                    out=oh_sb[:, e * NTW:(e + 1) * NTW],
                    in0=iota_bf[:],
                    scalar1=slot_sb[:, e:e + 1],
                    scalar2=we_sb[:, e:e + 1],
                    op0=mybir.AluOpType.is_equal,
                    op1=mybir.AluOpType.mult,
                )

            # feature-major activation buffers (bf16)
            fm = {
                "tx0": res.tile([F, SHARD], dt.bfloat16, name="fm_tx0"),
                "t1": res.tile([F, SHARD], dt.bfloat16, name="fm_t1"),
                "s2": res.tile([F, SHARD], dt.bfloat16, name="fm_s2"),
                "h": res.tile([F, SHARD], dt.bfloat16, name="fm_h"),
            }

            # node-major staging for table writes / transposes
            s_nm = res.tile([P, (NT // 2) * F], dt.bfloat16)

            # internal DRAM
            bounce = [dram.tile([SHARD, F], dt.bfloat16, name=f"bounce{i}") for i in range(3)]
            bounce_x = dram.tile([SHARD, F], dt.bfloat16, name="bounce_x")
            addr_space = "Local" if DBG_NO_AG else "Shared"
            ag = [dram.tile([NP_PAD, F], dt.bfloat16,
                            addr_space=addr_space, name=f"ag{i}")
                  for i in range(3)]
            ag_x = dram.tile([NP_PAD, F], dt.bfloat16,
                             addr_space=addr_space, name="ag_x")

            # ---------- helpers ----------
            def prop(table_ap, tag, probe=False):
                """one propagation: gather+scatter; results land in s_nm (node-major)."""
                with nc.named_scope(f"prop_{tag}"):
                    pr = None
                    if probe:
                        # tiny gpsimd-issued DMA touching the table: executes the
                        # collective-completion wait so the 1-wait-limited
                        # dynamic gathers below don't need it
                        pr = spool.tile([1, 2], dt.bfloat16, tag="pr")
                        nc.gpsimd.dma_start(out=pr[:], in_=table_ap.tensor[0:1, 0:2])
                    for nt in range(NT):
                        m_t = mpool.tile([P, TE * F], dt.bfloat16, tag="m")
                        # absorber: one strided gpsimd write touching each edge
                        # tile's corner carries the slot's WAR/WAW waits (and the
                        # table-probe dep for the first tile) so each 1-wait-
                        # limited dynamic gather below needs at most one wait.
                        # HW note: indirect DMA honors only ONE offset column
                        # per call, hence one gather per 128-edge tile.
                        if pr is not None and nt == 0:
                            nc.gpsimd.tensor_copy(out=m_t[0:1, 0:1], in_=pr[0:1, 0:1])
                        corner = m_t[:].rearrange("p (t f) -> p t f", f=F)[0:1, :, 0:1]
                        nc.gpsimd.memset(corner, 0)
                        for t in range(TE):
                            nc.gpsimd.indirect_dma_start(
                                out=m_t[:, t * F:(t + 1) * F],
                                out_offset=None,
                                in_=table_ap,
                                in_offset=bass.IndirectOffsetOnAxis(
                                    ap=src_sb[:, nt * TE + t:nt * TE + t + 1], axis=0),
                            )
                        ps = pscat.tile([NTW, F], dt.float32, space="PSUM", tag="ps")
                        for t in range(TE):
                            nc.tensor.matmul(
                                out=ps[:],
                                lhsT=oh_sb[:, (nt * TE + t) * NTW:(nt * TE + t + 1) * NTW],
                                rhs=m_t[:, t * F:(t + 1) * F],
                                start=(t == 0),
                                stop=(t == TE - 1),
                            )
                        j, b = nt // 2, nt % 2
                        nc.vector.tensor_copy(
                            out=s_nm[b * NTW:(b + 1) * NTW, j * F:(j + 1) * F],
                            in_=ps[:])

            def allgather(in_ap, out_tile, tag):
                with nc.named_scope(f"ag_{tag}"):
                    if DBG_NO_AG:
                        for r in range(C):
                            nc.sync.dma_start(
                                out=out_tile[r * SHARD:(r + 1) * SHARD, :],
                                in_=in_ap)
                    else:
                        nc.gpsimd.collective_compute(
                            "AllGather",
                            mybir.AluOpType.bypass,
                            replica_groups=rg,
                            ins=[in_ap],
                            outs=[out_tile.opt()],
                        )

            def table_write_and_ag(idx):
                """write s_nm -> bounce[idx] (node-major [SHARD, F]) and allgather."""
                bo = bounce[idx]
                view = bo[:].rearrange("(j p) f -> p j f", p=P)
                nc.sync.dma_start(out=view, in_=s_nm[:].rearrange("p (j f) -> p j f", f=F))
                allgather(bo[:], ag[idx], str(idx))

            def snm_to_fm(dest, tag):
                """transpose node-major s_nm into feature-major dest tile."""
                with nc.named_scope(f"fm_{tag}"):
                    for j in range(NT // 2):
                        pt = ptr.tile([F, P], dt.bfloat16, space="PSUM", tag="pt")
                        nc.tensor.transpose(out=pt[:], in_=s_nm[:, j * F:(j + 1) * F],
                                            identity=ident[:])
                        nc.vector.tensor_copy(out=dest[:, j * P:(j + 1) * P], in_=pt[:])

            def fm_to_snm(src_t, tag):
                """transpose feature-major tile back into s_nm node-major staging."""
                with nc.named_scope(f"nm_{tag}"):
                    for j in range(NT // 2):
                        pt = ptr.tile([P, F], dt.bfloat16, space="PSUM", tag="pt")
                        nc.tensor.transpose(out=pt[:], in_=src_t[:, j * P:(j + 1) * P],
                                            identity=ident[:F, :F])
                        nc.vector.tensor_copy(out=s_nm[:, j * F:(j + 1) * F], in_=pt[:])

            def dense(layer, tx0_t, t1_t, s2_t, h_t):
                """h = relu(tx0@W0' + t1@W1 + s2@W2') feature-major, bf16 out."""
                with nc.named_scope(f"dense_{layer}"):
                    wof = layer * 3 * F
                    nchunks = (SHARD + NCHUNK - 1) // NCHUNK
                    for ci in range(nchunks):
                        c0 = ci * NCHUNK
                        c1 = min(SHARD, c0 + NCHUNK)
                        pd = pdense.tile([F, NCHUNK], dt.float32, space="PSUM", tag="pd")
                        for ki, rhs_t in enumerate((tx0_t, t1_t, s2_t)):
                            nc.tensor.matmul(
                                out=pd[:, :c1 - c0],
                                lhsT=w_sb[:, wof + ki * F:wof + (ki + 1) * F],
                                rhs=rhs_t[:, c0:c1],
                                start=(ki == 0),
                                stop=(ki == 2),
                            )
                        nc.scalar.activation(
                            out=h_t[:, c0:c1], in_=pd[:, :c1 - c0],
                            func=mybir.ActivationFunctionType.Relu,
                            bias=bias_sb[:, layer:layer + 1],
                        )

            # ---------- pipeline ----------
            obs_t = res.tile([1, 1], dt.int32)
            nc.gpsimd.tensor_copy(out=obs_t[:], in_=src_sb[0:1, 0:1])

            # x: node-major into s_nm, transpose to feature-major, allgather table
            x_nm = bx_bf[0:XLEN].rearrange("(j p f) -> p j f", p=P, f=F)
            nc.sync.dma_start(out=s_nm[:].rearrange("p (j f) -> p j f", f=F),
                              in_=x_nm)
            # collectives may not read IO tensors: bounce x through internal DRAM
            x_nf = bx_bf[0:XLEN].rearrange("(n f) -> n f", f=F)
            nc.sync.dma_start(out=bounce_x[:], in_=x_nf)
            allgather(bounce_x[:], ag_x, "x")
            snm_to_fm(fm["tx0"], "tx0")

            # Layer 1
            prop(ag_x[:], "l1a", probe=True)          # s_nm = Tx1 own (node-major)
            table_write_and_ag(0)                     # ag[0] = Tx1 full
            snm_to_fm(fm["t1"], "t1")
            prop(ag[0][:], "l1b", probe=True)         # s_nm = L@Tx1 own
            snm_to_fm(fm["s2"], "s2")
            dense(0, fm["tx0"], fm["t1"], fm["s2"], fm["h"])
            fm_to_snm(fm["h"], "h1")
            table_write_and_ag(1)                     # ag[1] = h1 full

            # Layer 2
            prop(ag[1][:], "l2a", probe=True)
            table_write_and_ag(2)                     # ag[2] = Tx1' full
            snm_to_fm(fm["t1"], "t1b")
            prop(ag[2][:], "l2b", probe=True)
            snm_to_fm(fm["s2"], "s2b")
            dense(1, fm["h"], fm["t1"], fm["s2"], fm["tx0"])   # h2 -> fm["tx0"]

            # final linear [2 x SHARD]
            with nc.named_scope("final"):
                nchunks = (SHARD + NCHUNK - 1) // NCHUNK
                for ci in range(nchunks):
                    c0 = ci * NCHUNK
                    c1 = min(SHARD, c0 + NCHUNK)
                    pf = pdense.tile([2, NCHUNK], dt.float32, space="PSUM", tag="pd")
                    nc.tensor.matmul(out=pf[:, :c1 - c0], lhsT=wlin_sb[:],
                                     rhs=fm["tx0"][:, c0:c1], start=True, stop=True)
                    ot = spool.tile([2, NCHUNK], dt.float32, tag="ot")
                    nc.scalar.activation(
                        out=ot[:, :c1 - c0], in_=pf[:, :c1 - c0],
                        func=mybir.ActivationFunctionType.Identity,
                        bias=blin_sb[:],
                    )
                    nc.sync.dma_start(out=out_d[:, c0:c1], in_=ot[:, :c1 - c0])

    nc.compile()
    return nc


# --------------------------------------------------------------------------
# cached PJRT runner (jit built once; per-call = transfer + execute)
# --------------------------------------------------------------------------
def _get_runtime():
    global _RT
    if _RT is not None:
        return _RT
    import jax
    from jax.sharding import Mesh, PartitionSpec, NamedSharding
    from jax.experimental.shard_map import shard_map
    from concourse.bass2jax import (
        _bass_exec_p, install_neuronx_cc_hook, partition_id_tensor)

    nc = _build_kernel()
    install_neuronx_cc_hook()

    partition_name = nc.partition_id_tensor.name if nc.partition_id_tensor else None
    in_names, out_names, out_avals, zero_shapes = [], [], [], []
    for alloc in nc.m.functions[0].allocations:
        if not isinstance(alloc, mybir.MemoryLocationSet):
            continue
        name = alloc.memorylocations[0].name
        if alloc.kind == "ExternalInput":
            if name != partition_name:
                in_names.append(name)
        elif alloc.kind == "ExternalOutput":
            out_names.append(name)
            shape = tuple(alloc.tensor_shape)
            dtype = mybir.dt.np(alloc.dtype)
            out_avals.append(jax.core.ShapedArray(shape, dtype))
            zero_shapes.append((shape, dtype))
    n_params = len(in_names)
    in_names = in_names + out_names
    if partition_name is not None:
        in_names.append(partition_name)
    donate = tuple(range(n_params, n_params + len(out_names)))

    def _body(*args):
        operands = list(args)
        if partition_name is not None:
            operands.append(partition_id_tensor())
        outs = _bass_exec_p.bind(
            *operands, out_avals=tuple(out_avals), in_names=tuple(in_names),
            out_names=tuple(out_names), lowering_input_output_aliases=(),
            sim_require_finite=True, sim_require_nnan=True, nc=nc)
        return tuple(outs)

    devices = jax.devices()[:C]
    mesh = Mesh(np.asarray(devices), ("core",))
    in_specs = (PartitionSpec("core"),) * (n_params + len(out_names))
    out_specs = (PartitionSpec("core"),) * len(out_names)
    sharded = jax.jit(
        shard_map(_body, mesh=mesh, in_specs=in_specs, out_specs=out_specs,
                  check_rep=False),
        donate_argnums=donate, keep_unused=True)
    sh = NamedSharding(mesh, PartitionSpec("core"))

    _RT = dict(nc=nc, sharded=sharded, sh=sh, n_params=n_params,
               in_names=in_names, out_names=out_names,
               zero_shapes=zero_shapes, jax=jax)
    return _RT


# --------------------------------------------------------------------------
# entry point
# --------------------------------------------------------------------------
def kernel(x, edge_index, edge_weight, W1, b1, W2, b2, Wlin, blin,
           _trace=False, _tmpdir=None):
    import time as _time
    _t0 = _time.perf_counter()
    rt = _get_runtime()
    jax = rt["jax"]

    # donated output zeros: start the (tiny) transfer immediately
    (oshape, odtype) = rt["zero_shapes"][0]
    zeros = jax.device_put(
        np.zeros((C * oshape[0], *oshape[1:]), odtype), rt["sh"])

    x = np.asarray(x, np.float32)
    src = np.asarray(edge_index[0]).astype(np.int64, copy=False)
    dst = np.asarray(edge_index[1]).astype(np.int64, copy=False)
    w = np.asarray(edge_weight, np.float32)

    norm_w, new_id = _norm_and_perm(src, dst, w)

    # x blob: permuted, padded, bf16 -> ship ASAP (overlaps edge packing)
    xp = np.zeros((NP_PAD, F), np.float32)
    xp[new_id] = x
    xbf = xp.astype(BF)
    bx = xbf.reshape(C, XLEN).view(np.uint16)
    dev_x = jax.device_put(bx.reshape(-1), rt["sh"])

    # edge blob
    eb = _pack_edges(new_id[src], new_id[dst], norm_w)
    _fold_weights(np.asarray(W1, np.float32), np.asarray(b1, np.float32),
                  np.asarray(W2, np.float32), np.asarray(b2, np.float32),
                  np.asarray(Wlin, np.float32), np.asarray(blin, np.float32), eb)
    dev_e = jax.device_put(eb.reshape(-1), rt["sh"])

    out_arrs = rt["sharded"](dev_x, dev_e, zeros)
    res = np.asarray(out_arrs[0])                    # [C*2, SHARD] f32
    out_p = res.reshape(C, 2, SHARD).transpose(1, 0, 2).reshape(2, NP_PAD)
    out = out_p.T[new_id].astype(np.float32)         # [N, 2]
    kernel.last_spmd_wall_s = _time.perf_counter() - _t0
    return out
